# revision 1
# baseline (speedup 1.0000x reference)
"""Causal multi-head attention with RoPE for Trainium2, sharded over 8 NeuronCores.

Problem: B=4, T=2048, C=768, H=12, D=64, fp32.
    q,k,v = x @ wq/wk/wv  (per-head reshape), RoPE(q,k), causal softmax(q k^T/sqrt(D)) v,
    out = concat_heads @ wo.

Sharding: core c -> (batch b = c//2, head-group g = c%2 covering heads g*6..g*6+5).
Each core computes its 6 heads' attention and a partial output projection
y_c = out_heads(g) @ wo[rows g]; the host sums the two partials per batch.

On-core dataflow (all matmuls in float32r — full PE rate, ~1e-4 rel err):
  - host passes x^T so every matmul contracts along partitions.
  - q^T,k^T produced in [head_dim, T] layout (3 tiles of [128=2 heads, 2048]);
    RoPE applied via a block-rotation matmul (rotate_half) + cos/sin tensor ops.
  - scores computed transposed: S^T[k, q] = k^T.T @ q^T with K=64 row-pairing
    (even head at partitions 0:64, odd at 64:128 -> concurrent PE row groups).
  - P = exp(S/8) on ScalarE (batched over 2 PSUM banks); causal masking of
    diagonal tiles via gpsimd.affine_select (fill 0 after exp).
  - PV with a ones-row appended to V: out_unnorm^T[d, q] and l[q] in one
    accumulated matmul chain per (head, q-chunk).
  - softmax normalization: l row -> partition 0 (cross-quadrant copy),
    gpsimd.partition_broadcast, reciprocal_approx_fast, TT multiply.
  - output projection accumulates 3 head-pair chunks into [128, 768] PSUM.
"""

import numpy as np
from contextlib import ExitStack

B, T, C, H, D = 4, 2048, 768, 12, 64
HPC = 6          # heads per core
NP = 3           # head-pair tiles per core
CC = C // 128    # 6 contraction chunks
TT = T // 128    # 16 t tiles
QC = T // 512    # 4 q chunks
KC = T // 128    # 16 k chunks

_COMPILED = None


def _rope_tables():
    inv_freq = 1.0 / (10000.0 ** (np.arange(0, D, 2, dtype=np.float64) / D))  # [32]
    t = np.arange(T, dtype=np.float64)
    freqs = np.outer(t, inv_freq)                      # [T, 32]
    cosT = np.cos(freqs).T.astype(np.float32)          # [32, T]
    sinT = np.sin(freqs).T.astype(np.float32)
    ccat = np.tile(cosT, (4, 1))                       # [128, T]
    scat = np.tile(sinT, (4, 1))
    return np.ascontiguousarray(ccat), np.ascontiguousarray(scat)


def _rot_matrix():
    # rotate_half as a matmul: rot = R @ q (q in [D, T] layout), per 64-row block
    R = np.zeros((D, D), dtype=np.float32)
    R[0:32, 32:64] = -np.eye(32, dtype=np.float32)
    R[32:64, 0:32] = np.eye(32, dtype=np.float32)
    R2 = np.zeros((128, 128), dtype=np.float32)
    R2[0:64, 0:64] = R
    R2[64:128, 64:128] = R
    return np.ascontiguousarray(R2.T)                  # lhsT for out = R2 @ q


def _build_program():
    import concourse.tile as tile
    from concourse import bacc, mybir

    F32 = mybir.dt.float32
    F32R = mybir.dt.float32r
    BF16 = mybir.dt.bfloat16
    EXP = mybir.ActivationFunctionType.Exp

    nc = bacc.Bacc("TRN2", target_bir_lowering=False, debug=False, num_devices=8)

    xT_d = nc.dram_tensor("xT", [C, T], F32R, kind="ExternalInput").ap()
    wq_d = nc.dram_tensor("wq", [C, HPC * D], F32R, kind="ExternalInput").ap()
    wk_d = nc.dram_tensor("wk", [C, HPC * D], F32R, kind="ExternalInput").ap()
    wv_d = nc.dram_tensor("wv", [C, HPC * D], F32R, kind="ExternalInput").ap()
    wo_d = nc.dram_tensor("wo", [HPC * D, C], F32R, kind="ExternalInput").ap()
    ccat_d = nc.dram_tensor("ccat", [128, T], F32, kind="ExternalInput").ap()
    scat_d = nc.dram_tensor("scat", [128, T], F32, kind="ExternalInput").ap()
    r2t_d = nc.dram_tensor("r2t", [128, 128], F32R, kind="ExternalInput").ap()
    utri_d = nc.dram_tensor("utri", [128, 128], BF16, kind="ExternalInput").ap()
    eband_d = nc.dram_tensor("eband", [128, 128], BF16, kind="ExternalInput").ap()
    y_d = nc.dram_tensor("y", [T, C], F32, kind="ExternalOutput").ap()

    with tile.TileContext(nc) as tc, ExitStack() as ctx:
        big_pool = ctx.enter_context(tc.tile_pool(name="big", bufs=1))
        q_all = big_pool.tile([128, NP, T], F32R)
        k_all = big_pool.tile([128, NP, T], F32R)

        # ---- phase 0: input DMAs spread over both HWDGE rings ----
        cst_pool = ctx.enter_context(tc.tile_pool(name="cst", bufs=1))
        r2t = cst_pool.tile([128, 128], F32R)
        nc.sync.dma_start(r2t[:], r2t_d)
        utri = cst_pool.tile([128, 128], BF16)
        nc.scalar.dma_start(utri[:], utri_d)
        eband = cst_pool.tile([128, 128], BF16)
        nc.scalar.dma_start(eband[:], eband_d)
        wv_sb = cst_pool.tile([128, CC, HPC * D], F32R)
        nc.scalar.dma_start(wv_sb[:], wv_d.rearrange("(cc p) d -> p cc d", p=128))
        wo_sb = cst_pool.tile([128, NP, C], F32R)
        nc.scalar.dma_start(wo_sb[:], wo_d.rearrange("(hc p) c -> p hc c", p=128))

        xt_pool = ctx.enter_context(tc.tile_pool(name="xt", bufs=1))
        xt_sb = xt_pool.tile([128, CC, T], F32R)

        # ---- phase 1: q^T, k^T projections + RoPE ----
        with tc.tile_pool(name="w", bufs=1) as w_pool, \
             tc.tile_pool(name="const", bufs=1) as const_pool, \
             tc.tile_pool(name="p1ps", bufs=4, space="PSUM") as p1ps, \
             tc.tile_pool(name="p1tmp", bufs=2) as p1tmp:
            wq_sb = w_pool.tile([128, CC, HPC * D], F32R)
            nc.sync.dma_start(wq_sb[:], wq_d.rearrange("(cc p) d -> p cc d", p=128))
            wk_sb = w_pool.tile([128, CC, HPC * D], F32R)
            nc.sync.dma_start(wk_sb[:], wk_d.rearrange("(cc p) d -> p cc d", p=128))
            xT_r = xT_d.rearrange("(cc p) t -> p cc t", p=128)
            for cc in range(CC):
                nc.sync.dma_start(xt_sb[:, cc, :], xT_r[:, cc, :])
            ccat = const_pool.tile([128, T], F32)
            nc.scalar.dma_start(ccat[:], ccat_d)
            scat = const_pool.tile([128, T], F32)
            nc.scalar.dma_start(scat[:], scat_d)

            # HAM warmup while the input DMAs land
            warm_t = p1ps.tile([128, 1024], F32, tag="p1")
            warm = warm_t[:, 0:128]
            for _ in range(100):
                nc.tensor.matmul(warm[:], r2t[:], r2t[:], start=True, stop=True)

            for dt in range(NP):
                for w_sb, dst in ((wq_sb, q_all), (wk_sb, k_all)):
                    qraw = p1tmp.tile([128, T], F32R, tag="qraw")
                    ps_rot = [None, None]
                    for hh in range(2):
                        hsl = slice(hh * 1024, (hh + 1) * 1024)
                        ps_q = p1ps.tile([128, 1024], F32, tag="p1", name="ps_q")
                        for cc in range(CC):
                            for tq in range(2):
                                nc.tensor.matmul(
                                    ps_q[:, tq * 512:(tq + 1) * 512],
                                    w_sb[:, cc, dt * 128:(dt + 1) * 128],
                                    xt_sb[:, cc,
                                          hh * 1024 + tq * 512:
                                          hh * 1024 + (tq + 1) * 512],
                                    start=(cc == 0), stop=(cc == CC - 1),
                                )
                        nc.scalar.copy(qraw[:, hsl], ps_q[:, :])
                        ps_r = p1ps.tile([128, 1024], F32, tag="p1", name="ps_r")
                        for tq in range(2):
                            nc.tensor.matmul(
                                ps_r[:, tq * 512:(tq + 1) * 512],
                                r2t[:],
                                qraw[:, hh * 1024 + tq * 512:
                                      hh * 1024 + (tq + 1) * 512],
                                start=True, stop=True,
                            )
                        ps_rot[hh] = ps_r
                    nc.vector.tensor_mul(dst[:, dt, :], qraw[:].bitcast(F32),
                                         ccat[:])
                    for hh in range(2):
                        hsl = slice(hh * 1024, (hh + 1) * 1024)
                        nc.vector.tensor_mul(qraw[:, hsl], ps_rot[hh][:, :],
                                             scat[:, hsl])
                    nc.vector.tensor_add(dst[:, dt, :],
                                         dst[:, dt, :].bitcast(F32),
                                         qraw[:].bitcast(F32))

        # ---- phase 2: fused v-projection + attention + output projection ----
        # PSUM (8 banks): s0,s1 [128,1024] (4) + pv0,pv1 [65,512] (2) + aux (2)
        with tc.tile_pool(name="big2", bufs=1) as big2_pool, \
             tc.tile_pool(name="s_ps", bufs=1, space="PSUM") as s_psp, \
             tc.tile_pool(name="aux_ps", bufs=4, space="PSUM") as aux_psp, \
             tc.tile_pool(name="p_sb", bufs=2) as p_sbp, \
             tc.tile_pool(name="l_sb", bufs=2) as l_sbp, \
             tc.tile_pool(name="r_sb", bufs=2) as r_sbp, \
             tc.tile_pool(name="y_sb", bufs=2) as y_sbp:
            v_aug = big2_pool.tile([128, KC, HPC, D + 1], F32R)
            out_norm = big2_pool.tile([128, NP, T], F32R)
            nc.gpsimd.memset(v_aug[:, :, :, D:D + 1].bitcast(F32), 1.0)
            for qi, qc in enumerate((3, 2, 1, 0)):
                # all v chunks are needed by the first (largest) qc
                for tt in (range(KC) if qi == 0 else ()):
                    ps_v = aux_psp.tile([128, HPC * D], F32, tag="aux", name="ps_v")
                    for cc in range(CC):
                        nc.tensor.matmul(
                            ps_v[:, 0:HPC * D],
                            xt_sb[:, cc, tt * 128:(tt + 1) * 128],
                            wv_sb[:, cc, :],
                            start=(cc == 0), stop=(cc == CC - 1),
                        )
                    nc.vector.tensor_copy(
                        v_aug[:, tt, :, 0:D],
                        ps_v[:, 0:HPC * D].rearrange("p (h d) -> p h d", d=D),
                    )

                for p in range(NP):
                    nkc = 4 * qc + 4
                    pv = [aux_psp.tile([65, 512], F32, tag="aux", name=f"pv{h01}")
                          for h01 in (0, 1)]
                    for g0 in range(0, 4 * qc, 2):
                        kcs = list(range(g0, min(g0 + 2, 4 * qc)))
                        s_t = [s_psp.tile([128, 1024], F32, tag=f"s{h01}",
                                          name=f"s_t{h01}") for h01 in (0, 1)]
                        # alternate row groups so the K=64 pairs overlap on PE
                        for j, kc in enumerate(kcs):
                            for h01 in (0, 1):
                                r0, r1 = h01 * 64, h01 * 64 + 64
                                nc.tensor.matmul(
                                    s_t[h01][:, j * 512:(j + 1) * 512],
                                    k_all[r0:r1, p, kc * 128:(kc + 1) * 128],
                                    q_all[r0:r1, p, qc * 512:(qc + 1) * 512],
                                    start=True, stop=True,
                                )
                        for h01 in (0, 1):
                            pt = p_sbp.tile([128, 1024], F32R, tag=f"pt{h01}")
                            w = len(kcs) * 512
                            nc.scalar.activation(
                                pt[:, 0:w], s_t[h01][:, 0:w], EXP, scale=0.125,
                            )
                            for j, kc in enumerate(kcs):
                                nc.tensor.matmul(
                                    pv[h01][:],
                                    v_aug[:, kc, p * 2 + h01, :],
                                    pt[:, j * 512:(j + 1) * 512],
                                    start=(kc == 0), stop=False,
                                )
                    # diagonal tiles: A = j0(512)+j1(384), B = j2(256)+j3(128)
                    for half, segs in ((0, ((0, 0, 512), (1, 512, 384))),
                                       (1, ((2, 0, 256), (3, 256, 128)))):
                        s_d = [s_psp.tile([128, 1024], F32, tag=f"s{h01}",
                                          name=f"s_d{h01}") for h01 in (0, 1)]
                        for j, off, wj in segs:
                            kc = 4 * qc + j
                            for h01 in (0, 1):
                                r0, r1 = h01 * 64, h01 * 64 + 64
                                nc.tensor.matmul(
                                    s_d[h01][:, off:off + wj],
                                    k_all[r0:r1, p, kc * 128:(kc + 1) * 128],
                                    q_all[r0:r1, p,
                                          qc * 512 + 128 * j:qc * 512 + 512],
                                    start=True, stop=False,
                                )
                            for h01 in (0, 1):
                                nc.tensor.matmul(
                                    s_d[h01][:, off:off + 128],
                                    utri[:], eband[:],
                                    start=False, stop=True,
                                )
                        for h01 in (0, 1):
                            pt_d = p_sbp.tile([128, 1024], F32R, tag=f"pt{h01}",
                                              name="pt_d")
                            wtot = sum(sg[2] for sg in segs)
                            nc.scalar.activation(
                                pt_d[:, 0:wtot], s_d[h01][:, 0:wtot], EXP,
                                scale=0.125,
                            )
                            for j, off, wj in segs:
                                kc = 4 * qc + j
                                nc.tensor.matmul(
                                    pv[h01][:, 128 * j:512],
                                    v_aug[:, kc, p * 2 + h01, :],
                                    pt_d[:, off:off + wj],
                                    start=(kc == 0), stop=(j == 3),
                                )
                    for h01 in (0, 1):
                        lrow = l_sbp.tile([1, 512], F32, tag=f"l{h01}")
                        nc.vector.tensor_copy(lrow[0:1, :], pv[h01][64:65, :])
                        rbc = r_sbp.tile([64, 512], F32, tag=f"r{h01}")
                        nc.gpsimd.partition_broadcast(rbc[:], lrow[0:1, :],
                                                      channels=64)
                        nc.vector.reciprocal_approx_fast(rbc[:], rbc[:])
                        nc.vector.tensor_mul(
                            out_norm[h01 * 64:h01 * 64 + 64, p,
                                     qc * 512:(qc + 1) * 512],
                            pv[h01][0:64, :],
                            rbc[:],
                        )

                # output projection for this qc's four t-tiles
                for tt in range(4 * qc, 4 * qc + 4):
                    y_a = aux_psp.tile([128, 512], F32, tag="aux", name="y_a")
                    y_b = aux_psp.tile([128, 256], F32, tag="aux", name="y_b")
                    for hc in range(NP):
                        lhsT = out_norm[:, hc, tt * 128:(tt + 1) * 128]
                        nc.tensor.matmul(y_a[:, 0:512], lhsT, wo_sb[:, hc, 0:512],
                                         start=(hc == 0), stop=(hc == NP - 1))
                        nc.tensor.matmul(y_b[:, 0:256], lhsT, wo_sb[:, hc, 512:768],
                                         start=(hc == 0), stop=(hc == NP - 1))
                    yt = y_sbp.tile([128, C], F32, tag="yt")
                    nc.vector.tensor_copy(yt[:, 0:512], y_a[:, 0:512])
                    nc.vector.tensor_copy(yt[:, 512:768], y_b[:, 0:256])
                    nc.sync.dma_start(y_d[tt * 128:(tt + 1) * 128, :], yt[:])

    nc.compile()
    return nc


# make mybir importable inside _build_program's nested scopes
from concourse import mybir  # noqa: E402


def _get_compiled():
    global _COMPILED
    if _COMPILED is None:
        _COMPILED = _build_program()
    return _COMPILED


def _make_in_maps(inputs):
    x = np.asarray(inputs["x"], dtype=np.float32)
    wq = np.asarray(inputs["wq"], dtype=np.float32)
    wk = np.asarray(inputs["wk"], dtype=np.float32)
    wv = np.asarray(inputs["wv"], dtype=np.float32)
    wo = np.asarray(inputs["wo"], dtype=np.float32)

    import ml_dtypes

    ccat, scat = _rope_tables()
    r2t = _rot_matrix()
    m = np.arange(128)
    utri = (m[:, None] <= m[None, :]).astype(ml_dtypes.bfloat16)
    eband = np.zeros((128, 128), dtype=np.float32)
    eband[np.arange(1, 128), np.arange(127)] = -1e9
    eband = eband.astype(ml_dtypes.bfloat16)

    xTs = [np.ascontiguousarray(x[b].T) for b in range(B)]
    in_maps = []
    for c in range(8):
        b, g = c // 2, c % 2
        sl = slice(g * HPC * D, (g + 1) * HPC * D)
        in_maps.append(dict(
            xT=xTs[b],
            wq=np.ascontiguousarray(wq[:, sl]),
            wk=np.ascontiguousarray(wk[:, sl]),
            wv=np.ascontiguousarray(wv[:, sl]),
            wo=np.ascontiguousarray(wo[sl, :]),
            ccat=ccat, scat=scat, r2t=r2t, utri=utri, eband=eband,
        ))
    return in_maps


def kernel(x, wq, wk, wv, wo, mask):
    """Full inputs in, full output out. Shards across 8 NeuronCores internally.

    The mask input is the standard causal mask produced by setup_inputs();
    causality is implemented directly on-device.
    """
    from concourse.bass_utils import run_bass_kernel_spmd

    in_maps = _make_in_maps(dict(x=x, wq=wq, wk=wk, wv=wv, wo=wo))

    nc = _get_compiled()
    res = run_bass_kernel_spmd(nc, in_maps, list(range(8)))
    out = np.empty((B, T, C), dtype=np.float32)
    for b in range(B):
        out[b] = res.results[2 * b]["y"] + res.results[2 * b + 1]["y"]
    return out



# revision 3
# speedup vs baseline: 1.1130x; 1.1130x over previous
"""Causal multi-head attention with RoPE for Trainium2, sharded over 8 NeuronCores.

Problem: B=4, T=2048, C=768, H=12, D=64, fp32 in/out.
Sharding: core c -> (batch b = c//2, head-group g = c%2 covering heads g*6..g*6+5).
Each core computes its 6 heads' attention and a partial output projection; the
host sums the two partials per batch.

v2 design (vs fp32r baseline):
  - all matmul operands bf16 (FWL fast weight loads, half DMA/SBUF traffic);
    PSUM accumulation stays fp32.  Host pre-casts inputs to bf16.
  - software pipeline: projections+RoPE for head-pair p+1 are emitted
    interleaved with attention of pair p so TensorE stays dense (HAM warm);
    v-projection streams per q-chunk during pair 0; output projection per
    q-chunk after pair 2.
  - PSUM: s rotation 2x[128,1024]f32 (4 banks) + pv0/pv1 (2) + aux (2).
  - softmax: exp on ScalarE (f32 PSUM -> bf16 SBUF), N=1024 batches (both
    heads per kc chunk); ragged diagonal (widths 512/384 + 256/128) with
    post-exp upper-triangle zero-mask on VectorE (no mask matmuls).
  - RoPE: rotate via block-rotation matmul; combine = DVE copy + 2 DVE muls
    + GpSimd add (q_all += tsin) to keep DVE under budget.
  - l from ones-column appended to V (pv row 64); gpsimd partition_broadcast
    + reciprocal_approx_fast + DVE mul for normalization.
"""

import numpy as np
from contextlib import ExitStack

B, T, C, H, D = 4, 2048, 768, 12, 64
HPC = 6          # heads per core
NP = 3           # head-pair tiles per core
CC = C // 128    # 6 contraction chunks
TT = T // 128    # 16 t tiles
QC = T // 512    # 4 q chunks
KC = T // 128    # 16 k chunks

_COMPILED = None


def _rope_tables():
    inv_freq = 1.0 / (10000.0 ** (np.arange(0, D, 2, dtype=np.float64) / D))
    t = np.arange(T, dtype=np.float64)
    freqs = np.outer(t, inv_freq)                      # [T, 32]
    cosT = np.cos(freqs).T                             # [32, T]
    sinT = np.sin(freqs).T
    ccat = np.tile(cosT, (4, 1))                       # [128, T]
    scat = np.tile(sinT, (4, 1))
    return ccat, scat


def _rot_matrix():
    # rotate_half as a matmul: rot = R2 @ q (q in [d, t] layout), per 64-row block
    R = np.zeros((D, D), dtype=np.float32)
    R[0:32, 32:64] = -np.eye(32, dtype=np.float32)
    R[32:64, 0:32] = np.eye(32, dtype=np.float32)
    R2 = np.zeros((128, 128), dtype=np.float32)
    R2[0:64, 0:64] = R
    R2[64:128, 64:128] = R
    return np.ascontiguousarray(R2.T)                  # lhsT for out = R2 @ q


def _build_program():
    import concourse.tile as tile
    from concourse import bacc, mybir

    F32 = mybir.dt.float32
    BF16 = mybir.dt.bfloat16
    EXP = mybir.ActivationFunctionType.Exp

    nc = bacc.Bacc("TRN2", target_bir_lowering=False, debug=False, num_devices=8)

    xT_d = nc.dram_tensor("xT", [C, T], BF16, kind="ExternalInput").ap()
    wq_d = nc.dram_tensor("wq", [C, HPC * D], BF16, kind="ExternalInput").ap()
    wk_d = nc.dram_tensor("wk", [C, HPC * D], BF16, kind="ExternalInput").ap()
    wv_d = nc.dram_tensor("wv", [C, HPC * D], BF16, kind="ExternalInput").ap()
    wo_d = nc.dram_tensor("wo", [HPC * D, C], BF16, kind="ExternalInput").ap()
    ccat_d = nc.dram_tensor("ccat", [128, T], BF16, kind="ExternalInput").ap()
    scat_d = nc.dram_tensor("scat", [128, T], F32, kind="ExternalInput").ap()
    r2t_d = nc.dram_tensor("r2t", [128, 128], BF16, kind="ExternalInput").ap()
    tri_d = nc.dram_tensor("tri", [128, 128], BF16, kind="ExternalInput").ap()
    y_d = nc.dram_tensor("y", [T, C], F32, kind="ExternalOutput").ap()

    with tile.TileContext(nc) as tc, ExitStack() as ctx:
        big = ctx.enter_context(tc.tile_pool(name="big", bufs=1))
        q_all = big.tile([128, NP, T], BF16)
        k_all = big.tile([128, NP, T], BF16)
        out_norm = big.tile([128, NP, T], BF16)
        xt_sb = big.tile([128, CC, T], BF16)
        v_aug = big.tile([128, KC, HPC, D + 1], BF16)
        wq_sb = big.tile([128, CC, HPC * D], BF16)
        wk_sb = big.tile([128, CC, HPC * D], BF16)
        wv_sb = big.tile([128, CC, HPC * D], BF16)
        wo_sb = big.tile([128, NP, C], BF16)
        ccat = big.tile([128, T], BF16)
        scat = big.tile([128, T], F32)
        r2t = big.tile([128, 128], BF16)
        tri = big.tile([128, 128], BF16)

        # ---- input DMAs: interleave so first consumers unblock early ----
        nc.scalar.dma_start(r2t[:], r2t_d)
        nc.scalar.dma_start(tri[:], tri_d)
        xT_r = xT_d.rearrange("(cc p) t -> p cc t", p=128)
        nc.sync.dma_start(xt_sb[:, 0, :], xT_r[:, 0, :])
        nc.sync.dma_start(wq_sb[:], wq_d.rearrange("(cc p) d -> p cc d", p=128))
        nc.sync.dma_start(xt_sb[:, 1, :], xT_r[:, 1, :])
        nc.sync.dma_start(wk_sb[:], wk_d.rearrange("(cc p) d -> p cc d", p=128))
        for cc in range(2, CC):
            nc.sync.dma_start(xt_sb[:, cc, :], xT_r[:, cc, :])
        nc.scalar.dma_start(wv_sb[:], wv_d.rearrange("(cc p) d -> p cc d", p=128))
        nc.scalar.dma_start(ccat[:], ccat_d)
        nc.scalar.dma_start(scat[:], scat_d)
        nc.scalar.dma_start(wo_sb[:], wo_d.rearrange("(hc p) c -> p hc c", p=128))

        nc.gpsimd.memset(v_aug[:, :, :, D:D + 1], 1.0)

        # SBUF working pools (live for whole kernel)
        qraw_p = ctx.enter_context(tc.tile_pool(name="qraw", bufs=2))
        tsin_p = ctx.enter_context(tc.tile_pool(name="tsin", bufs=2))
        pt_p = ctx.enter_context(tc.tile_pool(name="pt", bufs=3))
        l_p = ctx.enter_context(tc.tile_pool(name="lr", bufs=2))
        y_p = ctx.enter_context(tc.tile_pool(name="yy", bufs=2))
        scr_p = ctx.enter_context(tc.tile_pool(name="scr", bufs=1))

        # ---------- work-unit builders (emission deferred via closures) ----
        def proj_unit(ps_pool, dt, w_sb, dst, tq):
            """Project + RoPE one [128, 512] t-slice of q or k for pair dt."""
            sl = slice(tq * 512, (tq + 1) * 512)
            ps_q = ps_pool.tile([128, 512], F32, tag="ps", name="ps_q")
            for cc in range(CC):
                nc.tensor.matmul(
                    ps_q[:], w_sb[:, cc, dt * 128:(dt + 1) * 128],
                    xt_sb[:, cc, sl], start=(cc == 0), stop=(cc == CC - 1),
                )
            qraw = qraw_p.tile([128, 512], BF16, tag="qraw")
            nc.vector.tensor_copy(qraw[:], ps_q[:])
            ps_r = ps_pool.tile([128, 512], F32, tag="ps", name="ps_r")
            nc.tensor.matmul(ps_r[:], r2t[:], qraw[:], start=True, stop=True)
            tsin = tsin_p.tile([128, 512], BF16, tag="tsin")
            nc.vector.tensor_mul(tsin[:], ps_r[:], scat[:, sl])
            nc.vector.tensor_mul(dst[:, dt, sl], qraw[:], ccat[:, sl])
            nc.gpsimd.tensor_add(dst[:, dt, sl], dst[:, dt, sl], tsin[:])

        def vproj_unit(ps_pool, tt):
            """Project one [128 t, 6 heads x 64] v tile into v_aug."""
            ps_v = ps_pool.tile([128, HPC * D], F32, tag="ps", name="ps_v")
            for cc in range(CC):
                nc.tensor.matmul(
                    ps_v[:], xt_sb[:, cc, tt * 128:(tt + 1) * 128],
                    wv_sb[:, cc, :], start=(cc == 0), stop=(cc == CC - 1),
                )
            nc.vector.tensor_copy(
                v_aug[:, tt, :, 0:D],
                ps_v[:].rearrange("p (h d) -> p h d", d=D),
            )

        # ---- stage 1: warmup + pair-0 proj + v chunks 0..3 ----
        with tc.tile_pool(name="s1ps", bufs=5, space="PSUM") as s1ps, \
             tc.tile_pool(name="dummy", bufs=1, space="PSUM") as dummy_p:
            warm = s1ps.tile([128, 512], F32, tag="ps", name="warm")
            for _ in range(90):
                nc.tensor.matmul(warm[:, 0:128], r2t[:], r2t[:],
                                 start=True, stop=True)
            # preload the exp table while PE warms up
            dummy = dummy_p.tile([128, 16], F32)
            nc.tensor.matmul(dummy[:], r2t[:], r2t[:, 0:16],
                             start=True, stop=True)
            scratch = scr_p.tile([128, 16], BF16)
            nc.scalar.activation(scratch[:], dummy[:], EXP, scale=0.125)

            units = [(w, d, tq) for tq in range(4)
                     for (w, d) in ((wq_sb, q_all), (wk_sb, k_all))]
            for i, (w, d, tq) in enumerate(units):
                if i % 2 == 0 and i // 2 < 4:
                    vproj_unit(s1ps, i // 2)
                proj_unit(s1ps, 0, w, d, tq)

        # ---- attention era ----
        with tc.tile_pool(name="s_ps", bufs=2, space="PSUM") as s_ps, \
             tc.tile_pool(name="pv_ps", bufs=1, space="PSUM") as pv_ps, \
             tc.tile_pool(name="aux_ps", bufs=2, space="PSUM") as aux_ps:

            def attention(p, qc, filler):
                """Attention for head-pair p, q-window qc*512..+512.
                `filler` is an iterator of zero-arg closures emitted between
                groups to keep TensorE dense (next pair's proj units etc)."""
                qsl = slice(qc * 512, (qc + 1) * 512)
                pv = [pv_ps.tile([128, 512], F32, tag=f"pv{h}", name=f"pv{h}")
                      for h in (0, 1)]

                def fill(n=1):
                    for _ in range(n):
                        u = next(filler, None)
                        if u is not None:
                            u()

                # off-diagonal kc chunks
                for kc in range(4 * qc):
                    s = s_ps.tile([128, 1024], F32, tag="s", name="s_od")
                    for h in (0, 1):
                        r0 = h * 64
                        nc.tensor.matmul(
                            s[:, h * 512:(h + 1) * 512],
                            k_all[r0:r0 + 64, p, kc * 128:(kc + 1) * 128],
                            q_all[r0:r0 + 64, p, qsl],
                            start=True, stop=True,
                        )
                    pt = pt_p.tile([128, 1024], BF16, tag="pt", name="pt_od")
                    nc.scalar.activation(pt[:], s[:], EXP, scale=0.125)
                    for h in (0, 1):
                        nc.tensor.matmul(
                            pv[h][0:65, :], v_aug[:, kc, 2 * p + h, :],
                            pt[:, h * 512:(h + 1) * 512],
                            start=(kc == 0), stop=False,
                        )
                    if kc % 3 == 2:
                        fill()

                # diagonal: 4 ragged chunks per head, post-exp triangle mask
                for h in (0, 1):
                    r0 = h * 64
                    ph = 2 * p + h
                    first = (qc == 0)
                    # segment A: j0 (w=512) + j1 (w=384)
                    dA = s_ps.tile([128, 1024], F32, tag="s", name="s_dA")
                    for j, off, w in ((0, 0, 512), (1, 512, 384)):
                        kc = 4 * qc + j
                        nc.tensor.matmul(
                            dA[:, off:off + w],
                            k_all[r0:r0 + 64, p, kc * 128:(kc + 1) * 128],
                            q_all[r0:r0 + 64, p,
                                  qc * 512 + 128 * j:(qc + 1) * 512],
                            start=True, stop=True,
                        )
                    ptA = pt_p.tile([128, 1024], BF16, tag="pt", name="pt_dA")
                    nc.scalar.activation(ptA[:, 0:896], dA[:, 0:896], EXP,
                                         scale=0.125)
                    nc.vector.tensor_mul(ptA[:, 0:128], ptA[:, 0:128], tri[:])
                    nc.vector.tensor_mul(ptA[:, 512:640], ptA[:, 512:640],
                                         tri[:])
                    nc.tensor.matmul(pv[h][0:65, 0:512],
                                     v_aug[:, 4 * qc, ph, :], ptA[:, 0:512],
                                     start=first, stop=False)
                    nc.tensor.matmul(pv[h][0:65, 128:512],
                                     v_aug[:, 4 * qc + 1, ph, :],
                                     ptA[:, 512:896], start=False, stop=False)
                    # segment B: j2 (w=256) + j3 (w=128)
                    dB = s_ps.tile([128, 1024], F32, tag="s", name="s_dB")
                    for j, off, w in ((2, 0, 256), (3, 256, 128)):
                        kc = 4 * qc + j
                        nc.tensor.matmul(
                            dB[:, off:off + w],
                            k_all[r0:r0 + 64, p, kc * 128:(kc + 1) * 128],
                            q_all[r0:r0 + 64, p,
                                  qc * 512 + 128 * j:(qc + 1) * 512],
                            start=True, stop=True,
                        )
                    ptB = pt_p.tile([128, 1024], BF16, tag="pt", name="pt_dB")
                    nc.scalar.activation(ptB[:, 0:384], dB[:, 0:384], EXP,
                                         scale=0.125)
                    nc.vector.tensor_mul(ptB[:, 0:128], ptB[:, 0:128], tri[:])
                    nc.vector.tensor_mul(ptB[:, 256:384], ptB[:, 256:384],
                                         tri[:])
                    nc.tensor.matmul(pv[h][0:65, 256:512],
                                     v_aug[:, 4 * qc + 2, ph, :],
                                     ptB[:, 0:256], start=False, stop=False)
                    nc.tensor.matmul(pv[h][0:65, 384:512],
                                     v_aug[:, 4 * qc + 3, ph, :],
                                     ptB[:, 256:384], start=False, stop=True)
                    fill()

                # softmax normalization -> out_norm
                for h in (0, 1):
                    r0 = h * 64
                    lrow = l_p.tile([1, 512], F32, tag=f"l{h}")
                    nc.vector.tensor_copy(lrow[0:1, :], pv[h][64:65, :])
                    rbc = l_p.tile([64, 512], F32, tag=f"r{h}")
                    nc.gpsimd.partition_broadcast(rbc[:], lrow[0:1, :],
                                                  channels=64)
                    nc.vector.reciprocal_approx_fast(rbc[:], rbc[:])
                    nc.vector.tensor_mul(
                        out_norm[r0:r0 + 64, p, qsl], pv[h][0:64, :], rbc[:],
                    )

            def outproj(tt):
                y_a = aux_ps.tile([128, 512], F32, tag="ps", name="y_a")
                y_b = aux_ps.tile([128, 256], F32, tag="ps", name="y_b")
                for hc in range(NP):
                    lhsT = out_norm[:, hc, tt * 128:(tt + 1) * 128]
                    nc.tensor.matmul(y_a[:], lhsT, wo_sb[:, hc, 0:512],
                                     start=(hc == 0), stop=(hc == NP - 1))
                    nc.tensor.matmul(y_b[:], lhsT, wo_sb[:, hc, 512:768],
                                     start=(hc == 0), stop=(hc == NP - 1))
                yt = y_p.tile([128, C], F32, tag="yt")
                nc.vector.tensor_copy(yt[:, 0:512], y_a[:])
                nc.vector.tensor_copy(yt[:, 512:768], y_b[:])
                nc.sync.dma_start(y_d[tt * 128:(tt + 1) * 128, :], yt[:])

            for p in range(NP):
                fillers = []
                if p < NP - 1:
                    fillers += [
                        (lambda w=w, d=d, tq=tq:
                         proj_unit(aux_ps, p + 1, w, d, tq))
                        for tq in range(4)
                        for (w, d) in ((wq_sb, q_all), (wk_sb, k_all))
                    ]
                filler = iter(fillers)
                for qc in range(QC):
                    if p == 0 and qc >= 1:
                        for tt in range(4 * qc, 4 * qc + 4):
                            vproj_unit(aux_ps, tt)
                    attention(p, qc, filler)
                    if p == NP - 1:
                        for tt in range(4 * qc, 4 * qc + 4):
                            outproj(tt)
                # drain any leftover proj units
                for u in filler:
                    u()

    nc.compile()
    return nc


def _get_compiled():
    global _COMPILED
    if _COMPILED is None:
        _COMPILED = _build_program()
    return _COMPILED


def _make_in_maps(inputs):
    import ml_dtypes

    BF = ml_dtypes.bfloat16
    x = np.asarray(inputs["x"], dtype=np.float32)
    wq = np.asarray(inputs["wq"], dtype=np.float32)
    wk = np.asarray(inputs["wk"], dtype=np.float32)
    wv = np.asarray(inputs["wv"], dtype=np.float32)
    wo = np.asarray(inputs["wo"], dtype=np.float32)

    ccat, scat = _rope_tables()
    ccat_b = np.ascontiguousarray(ccat.astype(BF))
    scat_f = np.ascontiguousarray(scat.astype(np.float32))
    r2t = np.ascontiguousarray(_rot_matrix().astype(BF))
    m = np.arange(128)
    tri = np.ascontiguousarray(
        (m[:, None] <= m[None, :]).astype(BF))       # keep col >= row

    xTs = [np.ascontiguousarray(x[b].T.astype(BF)) for b in range(B)]
    in_maps = []
    for c in range(8):
        b, g = c // 2, c % 2
        sl = slice(g * HPC * D, (g + 1) * HPC * D)
        in_maps.append(dict(
            xT=xTs[b],
            wq=np.ascontiguousarray(wq[:, sl].astype(BF)),
            wk=np.ascontiguousarray(wk[:, sl].astype(BF)),
            wv=np.ascontiguousarray(wv[:, sl].astype(BF)),
            wo=np.ascontiguousarray(wo[sl, :].astype(BF)),
            ccat=ccat_b, scat=scat_f, r2t=r2t, tri=tri,
        ))
    return in_maps


def kernel(x, wq, wk, wv, wo, mask):
    """Full inputs in, full output out. Shards across 8 NeuronCores internally.

    The mask input is the standard causal mask produced by setup_inputs();
    causality is implemented directly on-device.
    """
    from concourse.bass_utils import run_bass_kernel_spmd

    in_maps = _make_in_maps(dict(x=x, wq=wq, wk=wk, wv=wv, wo=wo))

    nc = _get_compiled()
    res = run_bass_kernel_spmd(nc, in_maps, list(range(8)))
    out = np.empty((B, T, C), dtype=np.float32)
    for b in range(B):
        out[b] = res.results[2 * b]["y"] + res.results[2 * b + 1]["y"]
    return out
